# revision 15
# baseline (speedup 1.0000x reference)
"""Trainium2 Bass kernel for nn_AggregationRebuild_HN (sparse_attention).

Computes, for each of B=512 samples:
    out[b] = sum_j softmax(sim[b] / 0.02)[j] * block_j(b)          # [64, 128]
where block_j(b) are 3 "positive" rows (512 + 3b + j of p_enc_out) and 16
gathered "negative" rows (p_enc_out[negative_index[b, j]]).

Strategy ("scatter-softmax-matmul"):
  * Shard the P*D = 8192 feature axis across 8 cores (1024 features each).
    Every core reads its slice of p_enc_out exactly once (~8.5 MiB) -- the
    roofline-minimal HBM traffic -- instead of re-reading gathered rows.
  * The whole gather + weighted sum becomes a single PE-matmul chain per
    output tile: out[b, :] = sum_k WT[k, b] * pool[k, :], with the K axis
    made of 4 chunks of original rows (negatives), >=1 "replica" chunk
    (extra copies of pool rows for duplicate indices within a sample), and
    3 per-tile chunks of positive rows (block-band structure).
  * WT (the softmax *numerators* scattered into K-slot positions) is
    produced on device by one ACT Exp over a host-built scatter of the
    max-shifted logits (empty slots hold -3e4 -> exp -> 0).  The softmax
    denominator is computed on device from the [512, 19] logits; 1/Z lands
    as a per-partition scale on the PSUM->SBUF copy.
  * Host-side work is index bookkeeping + the standard stable-softmax max
    shift only; exp/sum/normalize and all matvec math run on device.

Hardware constraint that shapes the code: most TRN2 instructions accept a
single sync-wait command, so the program is arranged so every instruction
needs at most one cross-engine wait (dummy "wait absorber" ops pre-observe
sems; DMAs are consolidated so semaphore lanes are not reused and the tail
drain's wait list stays small).
"""

from contextlib import ExitStack

import numpy as np

_B = 512            # bs * n_vars
_P = 64             # patch_num
_D = 128            # d_model
_KP = 3             # k_positive
_KN = 16            # k_negative
_NCORES = 8
_PPC = _P // _NCORES        # patches per core = 8
_PDC = _PPC * _D            # features per core = 1024
_SENT = -3.0e4              # empty-slot sentinel; exp(50 * -3e4) == 0
_SCALE = 50.0               # 1 / temperature
_NTILES = _B // 128         # 4 M-tiles of 128 samples


def _build_host(sim, neg_idx):
    """Index bookkeeping + stable-softmax shift.

    Returns (ssc_ext, rep_rows, n_rep_chunks):
      ssc_ext [128, ncols + 76] f32:
        cols [512c, 512c+512) c<4   : original-row slots (chunk c)
        cols [2048 + 512q, ...)     : replica chunks q
        cols [pos_col0 + 128*(3t+pc), +128): positive band block (t, pc)
        cols [ncols, ncols+76)      : max-shifted logits, [p, t, k] layout
      rep_rows [128 * n_rep_chunks] : pool row content of each replica slot
    """
    sim = np.asarray(sim, np.float32)
    neg_idx = np.asarray(neg_idx).astype(np.int64)
    m = sim.max(axis=1, keepdims=True)
    simsh = np.ascontiguousarray(sim - m, dtype=np.float32)  # [B, 19]

    # Duplicate handling: the d-th extra occurrence of pool row r within one
    # sample needs its own K slot whose rhs content is (a copy of) row r.
    occ = {}
    dup_rank = np.zeros((_B, _KN), np.int64)
    for b in range(_B):
        seen = {}
        for j in range(_KN):
            r = int(neg_idx[b, j])
            dup_rank[b, j] = seen.get(r, 0)
            seen[r] = seen.get(r, 0) + 1
        for r, c in seen.items():
            if c - 1 > occ.get(r, 0):
                occ[r] = c - 1
    rep_slot = {}
    rep_rows = []
    for r in sorted(occ):
        for d in range(occ[r]):
            rep_slot[(r, d)] = len(rep_rows)
            rep_rows.append(r)
    n_rep_chunks = max(1, -(-len(rep_rows) // 128))
    rep_rows = rep_rows + [0] * (n_rep_chunks * 128 - len(rep_rows))

    pos_col0 = 2048 + 512 * n_rep_chunks
    ncols = pos_col0 + 128 * 3 * _NTILES
    ssc_ext = np.full((128, ncols + _NTILES * 19), _SENT, np.float32)

    for b in range(_B):
        t, bl = b >> 7, b & 127
        for j in range(_KP):
            slot = 3 * bl + j
            pc, kl = slot >> 7, slot & 127
            ssc_ext[kl, pos_col0 + 128 * (3 * t + pc) + bl] = simsh[b, j]
        for j in range(_KN):
            r = int(neg_idx[b, j])
            d = int(dup_rank[b, j])
            if d == 0:
                ssc_ext[r & 127, 512 * (r >> 7) + b] = simsh[b, _KP + j]
            else:
                q = rep_slot[(r, d - 1)]
                ssc_ext[q & 127, 2048 + 512 * (q >> 7) + b] = simsh[b, _KP + j]
    # shifted logits for the softmax denominator: [p, t, k] layout
    ssc_ext[:, ncols:] = simsh.reshape(_NTILES, 128, 19).transpose(1, 0, 2).reshape(128, -1)
    return ssc_ext, np.array(rep_rows, np.int64), n_rep_chunks


def _kernel_body(ctx, tc, out_ap, pool_ap, ssc_ap, n_rep_chunks):
    import concourse.mybir as mybir

    nc = tc.nc
    f32 = mybir.dt.float32
    AF = mybir.ActivationFunctionType
    n_chunks = 16 + n_rep_chunks
    pos_col0 = 2048 + 512 * n_rep_chunks
    ncols = pos_col0 + 128 * 3 * _NTILES

    const = ctx.enter_context(tc.tile_pool(name="const", bufs=1))
    psum_pool = ctx.enter_context(tc.tile_pool(name="psum", bufs=8, space="PSUM"))

    # --- scattered logits (+ appended plain logits), one DMA ------------
    ssc = const.tile([128, ncols + _NTILES * 19], f32, tag="ssc")
    nc.sync.dma_start(out=ssc[:], in_=ssc_ap[:, :])

    # softmax numerators, scattered into K-slot positions
    wt = const.tile([128, ncols], f32, tag="wt")
    nc.scalar.activation(out=wt[:], in_=ssc[:, :ncols], func=AF.Exp, scale=_SCALE)

    # softmax denominator Z[b] and 1/Z
    rz = const.tile([128, _NTILES], f32, tag="rz")
    for t in range(_NTILES):
        e = const.tile([128, 19], f32, tag=f"e{t}", name=f"e{t}")
        nc.scalar.activation(
            out=e[:],
            in_=ssc[:, ncols + 19 * t : ncols + 19 * (t + 1)],
            func=AF.Exp,
            scale=_SCALE,
        )
        z = const.tile([128, 1], f32, tag=f"z{t}", name=f"z{t}")
        nc.vector.reduce_sum(out=z[:], in_=e[:], axis=mybir.AxisListType.X)
        nc.vector.reciprocal(out=rz[:, t : t + 1], in_=z[:])

    # Wait-absorbers: most instructions can carry one sync-wait, so
    # pre-observe cross-engine sems with cheap ops (PE observes ACT's wt
    # write; ACT observes DVE's rz write).
    dps = psum_pool.tile([1, 1], f32, tag="dps", bufs=1)
    nc.tensor.matmul(dps[:], lhsT=wt[:1, 0:1], rhs=wt[:1, 0:1], start=True, stop=True)
    rz_touch = const.tile([128, _NTILES], f32, tag="rz_touch")
    nc.scalar.copy(out=rz_touch[:], in_=rz[:])

    # --- pool: one wide tile, 3 DMAs (chunk c = cols [1024c, 1024c+1024)) --
    # DRAM row order (host-arranged): negatives (chunks 0-3), replicas
    # (chunks 4..4+R-1), positives (chunks 4+R..15+R).  Three groups keep the
    # tail drain's wait list within the HW limit while still letting the
    # first matmul chains start after ~2/3 of the load.
    pool_sb = const.tile([128, n_chunks * _PDC], f32, tag="pool_sb")
    pool_view = pool_ap.rearrange("(c p) n -> c p n", p=128)

    def chunk(k):
        return pool_sb[:, _PDC * k : _PDC * (k + 1)]

    nr = n_rep_chunks
    bounds = [0, 4 + nr, 4 + nr + 6, n_chunks]
    for k0, k1 in zip(bounds[:-1], bounds[1:]):
        nc.sync.dma_start(
            out=pool_sb[:, _PDC * k0 : _PDC * k1].rearrange(
                "p (c n) -> p c n", n=_PDC
            ),
            in_=pool_view[k0:k1].rearrange("c p n -> p c n"),
        )

    # --- matmul chains + 1/Z scale ---------------------------------------
    out_sb = const.tile([128, _NTILES * _PDC], f32, tag="out_sb")
    for t in range(_NTILES):
        for h in range(2):
            ps = psum_pool.tile([128, 512], f32, tag="ps", bufs=7, name=f"ps{t}{h}")
            chain = [(512 * c + 128 * t, c) for c in range(4)]
            chain += [(2048 + 512 * q + 128 * t, 4 + q) for q in range(n_rep_chunks)]
            chain += [
                (pos_col0 + 128 * (3 * t + pc), 4 + n_rep_chunks + 3 * t + pc)
                for pc in range(3)
            ]
            for i, (wc, pk) in enumerate(chain):
                nc.tensor.matmul(
                    ps[:],
                    lhsT=wt[:, wc : wc + 128],
                    rhs=chunk(pk)[:, 512 * h : 512 * (h + 1)],
                    start=(i == 0),
                    stop=(i == len(chain) - 1),
                )
            nc.scalar.activation(
                out=out_sb[:, _PDC * t + 512 * h : _PDC * t + 512 * (h + 1)],
                in_=ps[:],
                func=AF.Copy,
                scale=rz[:, t : t + 1],
            )
    # single consolidated store (SWDGE: fresh sem lane -> one wait)
    nc.gpsimd.dma_start(
        out=out_ap.rearrange("(t p) n -> p t n", p=128),
        in_=out_sb[:].rearrange("p (t n) -> p t n", n=_PDC),
    )


_prog_cache = {}


def _get_program(n_rep_chunks):
    if n_rep_chunks in _prog_cache:
        return _prog_cache[n_rep_chunks]
    import concourse.bacc as bacc
    import concourse.mybir as mybir
    import concourse.tile as tile

    nc = bacc.Bacc(
        "TRN2",
        target_bir_lowering=False,
        debug=False,
        enable_asserts=False,
        num_devices=_NCORES,
    )
    n_pool_rows = 2048 + 128 * n_rep_chunks
    pos_col0 = 2048 + 512 * n_rep_chunks
    ncols = pos_col0 + 128 * 3 * _NTILES
    f32 = mybir.dt.float32
    pool_ap = nc.dram_tensor("pool", [n_pool_rows, _PDC], f32, kind="ExternalInput").ap()
    ssc_ap = nc.dram_tensor(
        "ssc", [128, ncols + _NTILES * 19], f32, kind="ExternalInput"
    ).ap()
    out_ap = nc.dram_tensor("out", [_B, _PDC], f32, kind="ExternalOutput").ap()
    with tile.TileContext(nc) as tc:
        with ExitStack() as ctx:
            _kernel_body(ctx, tc, out_ap, pool_ap, ssc_ap, n_rep_chunks)
    nc.compile()
    _prog_cache[n_rep_chunks] = nc
    return nc


def _prepare(similarity_matrix, p_enc_out, negative_index):
    sim = np.asarray(similarity_matrix, np.float32)
    pool = np.asarray(p_enc_out, np.float32)
    assert sim.shape == (_B, _KP + _KN), sim.shape
    assert pool.shape == (_B * (1 + _KP), _P, _D), pool.shape
    ssc_ext, rep_rows, n_rep_chunks = _build_host(sim, negative_index)
    in_maps = []
    for c in range(_NCORES):
        sl = pool[:, _PPC * c : _PPC * (c + 1), :].reshape(-1, _PDC)
        rep = pool[rep_rows, _PPC * c : _PPC * (c + 1), :].reshape(-1, _PDC)
        in_maps.append(
            {
                # row order: negatives, replicas, positives
                "pool": np.ascontiguousarray(
                    np.concatenate([sl[:_B], rep, sl[_B:]], axis=0)
                ),
                "ssc": ssc_ext,
            }
        )
    return in_maps, n_rep_chunks


def _postprocess(results):
    outs = [r["out"].reshape(_B, _PPC, _D) for r in results]
    return np.ascontiguousarray(np.concatenate(outs, axis=1))


def kernel(similarity_matrix, p_enc_out, negative_index, **_unused):
    from concourse.bass_utils import run_bass_kernel_spmd

    in_maps, n_rep_chunks = _prepare(similarity_matrix, p_enc_out, negative_index)
    nc = _get_program(n_rep_chunks)
    res = run_bass_kernel_spmd(nc, in_maps, core_ids=list(range(_NCORES)))
    return _postprocess(res.results)


if __name__ == "__main__":
    # smoke test with random data (no reference available here)
    rng = np.random.default_rng(0)
    sim = rng.standard_normal((_B, _KP + _KN), dtype=np.float32)
    pool = rng.standard_normal((_B * (1 + _KP), _P, _D), dtype=np.float32)
    idx = rng.integers(0, _B, size=(_B, _KN))
    out = kernel(similarity_matrix=sim, p_enc_out=pool, negative_index=idx)
    print("out", out.shape, out.dtype, float(np.abs(out).mean()))
